# revision 7
# baseline (speedup 1.0000x reference)
"""Trainium2 Bass kernel for nn_BaselineModel_47682726921062.

Model: token embedding lookup -> input projection -> 512-step tanh RNN
-> softmax over the hidden dim. Output [64, 512, 512] = softmax(h, axis=1)
with h[b, :, t] the hidden state after step t.

Strategy: data-parallel over batch across 8 NeuronCores (8 examples/core),
weights replicated, zero collectives.

v2 over the v1 baseline:
  - xp (input projection + bias) is accumulated DIRECTLY into PSUM via
    matmuls (bias by outer-product MM with a first-row-ones moving tile),
    and the recurrence matmuls accumulate onto it (start=False). This
    removes the per-step VectorE add from the serial chain entirely:
    per step only [16 MMs] -> Tanh -> [16 MMs] remains.
  - the sequence is processed in 16 segments of 32 steps; PSUM holds xp
    for 3 segments (2 banks each, 6 of 8 banks) so gather, projection
    (seg s+1) and softmax+output (seg s-1) all run in the engine idle
    slots under the recurrence of seg s.
  - numerics: embedding rows are gathered as host-precomputed (hi, lo)
    bf16 pairs (one 2KB-elem transposing SWDGE gather), W_ih applied as
    hi/lo bf16 pair (3 cross terms), bias as hi/lo pair, softmax wholly
    in fp32 from a second PSUM read (tanh_f32 -> exp -> ones-matmul sums
    -> DVE reciprocal -> fp32 normalize). Only W_hh and the recurrent h
    stay plain bf16.
"""

import sys

if "/opt/trn_rl_repo" not in sys.path:
    sys.path.insert(0, "/opt/trn_rl_repo")

import numpy as np
import ml_dtypes

BATCH, SEQ, VOCAB, DIM = 64, 512, 32000, 512
NCORES = 8
BC = BATCH // NCORES          # 8 examples per core
P = 128
KC = DIM // P                 # 4 chunks of 128
NIDX = SEQ * BC               # 4096 gathered rows per core
NBLK = 8                      # gather blocks of 512 (t,b) rows
BLK = NIDX // NBLK            # 512
SEG = 32                      # recurrence steps per segment
NSEG = SEQ // SEG             # 16
SCOL = SEG * BC               # 256 psum columns per segment

TRACE = False
LAST_RESULT = None

_cache = {}


def _build():
    import concourse.mybir as mybir
    import concourse.tile as tile
    from concourse import bacc

    f32 = mybir.dt.float32
    bf16 = mybir.dt.bfloat16
    Act = mybir.ActivationFunctionType

    nc = bacc.Bacc("TRN2")

    embp = nc.dram_tensor("embp", [VOCAB, 2 * DIM], bf16, kind="ExternalInput")
    idx = nc.dram_tensor("idx", [P, NIDX // 16], mybir.dt.int16, kind="ExternalInput")
    wih_hi = nc.dram_tensor("wih_hi", [DIM, DIM], bf16, kind="ExternalInput")  # W_ih.T hi
    wih_lo = nc.dram_tensor("wih_lo", [DIM, DIM], bf16, kind="ExternalInput")  # W_ih.T lo
    whh = nc.dram_tensor("whh", [DIM, DIM], bf16, kind="ExternalInput")        # W_hh.T
    # biasm[k, bank, m]: rows 0/1 = bias_hi for the bank's two mc halves,
    # rows 2/3 = bias_lo; one N=512 outer-product MM per PSUM bank writes
    # bias over the whole bank (the only start=True MM the bank ever sees).
    biasm = nc.dram_tensor("biasm", [P, 2, P], bf16, kind="ExternalInput")
    e0 = nc.dram_tensor("e0", [P, 2 * SCOL], bf16, kind="ExternalInput")
    ones = nc.dram_tensor("ones", [P, P], f32, kind="ExternalInput")
    out = nc.dram_tensor("out", [BC, DIM, SEQ], f32, kind="ExternalOutput")

    with tile.TileContext(nc) as tc:
        with (
            tc.tile_pool(name="consts", bufs=1) as consts,
            tc.tile_pool(name="xe", bufs=2) as xe_pool,
            tc.tile_pool(name="h", bufs=1) as h_pool,
            tc.tile_pool(name="tf", bufs=2) as tf_pool,
            tc.tile_pool(name="ex", bufs=2) as ex_pool,
            tc.tile_pool(name="rc", bufs=2) as rc_pool,
            tc.tile_pool(name="st", bufs=2) as st_pool,
            tc.tile_pool(name="xp", bufs=3, space="PSUM") as xp_pool,
            tc.tile_pool(name="sm", bufs=2, space="PSUM") as sm_pool,
        ):
            idx_sb = consts.tile([P, NIDX // 16], mybir.dt.int16)
            nc.sync.dma_start(idx_sb[:], idx[:])
            wih_hi_sb = consts.tile([P, KC, DIM], bf16)
            nc.sync.dma_start(wih_hi_sb[:], wih_hi.rearrange("(kc p) m -> p kc m", p=P))
            wih_lo_sb = consts.tile([P, KC, DIM], bf16)
            nc.sync.dma_start(wih_lo_sb[:], wih_lo.rearrange("(kc p) m -> p kc m", p=P))
            whh_sb = consts.tile([P, KC, DIM], bf16)
            nc.sync.dma_start(whh_sb[:], whh.rearrange("(kc p) m -> p kc m", p=P))
            biasm_sb = consts.tile([P, 2, P], bf16)
            nc.sync.dma_start(biasm_sb[:], biasm[:])
            e0_sb = consts.tile([P, 2 * SCOL], bf16)
            nc.sync.dma_start(e0_sb[:], e0[:])
            ones_sb = consts.tile([P, P], f32)
            nc.sync.dma_start(ones_sb[:], ones[:])

            hT_all = h_pool.tile([P, SEQ, KC, BC], bf16)   # 32 KB/partition

            xe_blocks = {}

            def emit_gather(nb):
                xe = xe_pool.tile([P, 2 * KC, BLK], bf16, tag="xe")
                nc.gpsimd.dma_gather(
                    xe[:], embp[:], idx_sb[:, nb * 32 : (nb + 1) * 32],
                    num_idxs=BLK, num_idxs_reg=BLK, elem_size=2 * DIM,
                    transpose=True,
                )
                xe_blocks[nb] = xe

            def proj_thunks(s):
                """One thunk per matmul for segment s's xp preload."""
                nb, half = s // 2, s % 2
                col = slice(half * SCOL, (half + 1) * SCOL)
                state = {}

                def alloc():
                    state["ps"] = xp_pool.tile([P, KC, SCOL], f32, tag="xp", name="xp")

                def bias_mm(bank):
                    if bank == 0:
                        alloc()
                    # one start=True MM covering the entire PSUM bank
                    nc.tensor.matmul(
                        state["ps"][:, 2 * bank : 2 * bank + 2, :].rearrange(
                            "p c n -> p (c n)"
                        ),
                        biasm_sb[:, bank, :],
                        e0_sb[:],
                        start=True, stop=False, skip_group_check=True,
                    )

                thunks = [lambda: bias_mm(0), lambda: bias_mm(1)]
                for mc in range(KC):
                    msl = slice(mc * P, (mc + 1) * P)
                    for kc in range(KC):
                        for w_sb, xc in (
                            (wih_hi_sb, kc),          # Whi * xe_hi
                            (wih_hi_sb, KC + kc),     # Whi * xe_lo
                            (wih_lo_sb, kc),          # Wlo * xe_hi
                        ):
                            def pmm(mc=mc, msl=msl, kc=kc, w_sb=w_sb, xc=xc):
                                nc.tensor.matmul(
                                    state["ps"][:, mc, :],
                                    w_sb[:, kc, msl],
                                    xe_blocks[nb][:, xc, col],
                                    start=False, stop=False,
                                    skip_group_check=True,
                                )
                            thunks.append(pmm)
                return state, thunks

            def softmax_thunks(s, ps, st_state):
                """Softmax + output for segment s, reading its psum tile."""
                thunks = []
                state = {}

                def mk_tf(mc):
                    if mc == 0:
                        state["tf"] = tf_pool.tile([P, KC, SCOL], f32, tag="tf", name="tf")
                    nc.scalar.activation(state["tf"][:, mc, :], ps[:, mc, :], Act.Tanh)

                def mk_ex(mc):
                    if mc == 0:
                        state["ex"] = ex_pool.tile([P, KC, SCOL], f32, tag="ex", name="ex")
                    nc.scalar.activation(state["ex"][:, mc, :], state["tf"][:, mc, :], Act.Exp)

                def mk_sum(mc):
                    if mc == 0:
                        state["sp"] = sm_pool.tile([P, SCOL], f32, tag="sum", name="sp")
                    nc.tensor.matmul(
                        state["sp"][:], ones_sb[:], state["ex"][:, mc, :],
                        start=(mc == 0), stop=(mc == KC - 1),
                        skip_group_check=True,
                    )

                def mk_recip():
                    state["rc"] = rc_pool.tile([P, SEG, BC], f32, tag="rc", name="rc")
                    nc.vector.reciprocal(
                        state["rc"][:], state["sp"][:].rearrange("p (t b) -> p t b", b=BC)
                    )

                def mk_norm(mc):
                    if mc == 0 and s % 2 == 0:
                        st_state["st"] = st_pool.tile([P, KC, BC, 2 * SEG], f32, tag="st", name="st")
                    toff = (s % 2) * SEG
                    nc.vector.tensor_tensor(
                        st_state["st"][:, mc, :, toff : toff + SEG].rearrange(
                            "p b t -> p t b"
                        ),
                        state["ex"][:, mc, :].rearrange("p (t b) -> p t b", b=BC),
                        state["rc"][:],
                        mybir.AluOpType.mult,
                    )

                def mk_dma(mc):
                    ts0 = (s // 2) * 2 * SEG
                    nc.sync.dma_start(
                        out[:, mc * P : (mc + 1) * P, ts0 : ts0 + 2 * SEG].rearrange(
                            "b p t -> p b t"
                        ),
                        st_state["st"][:, mc],
                    )

                for mc in range(KC):
                    thunks.append(lambda mc=mc: mk_tf(mc))
                for mc in range(KC):
                    thunks.append(lambda mc=mc: mk_ex(mc))
                for mc in range(KC):
                    thunks.append(lambda mc=mc: mk_sum(mc))
                thunks.append(mk_recip)
                for mc in range(KC):
                    thunks.append(lambda mc=mc: mk_norm(mc))
                if s % 2 == 1:
                    for mc in range(KC):
                        thunks.append(lambda mc=mc: mk_dma(mc))
                return thunks

            # ---- prologue -------------------------------------------------
            emit_gather(0)
            emit_gather(1)
            st_state = {}
            proj_state, thunks0 = proj_thunks(0)
            for th in thunks0:
                th()
            seg_ps = {0: proj_state}

            pending_proj = []
            pending_soft = []

            with nc.named_scope("recurrence"):
                for s in range(NSEG):
                    # stage work that runs under this segment's recurrence
                    if s >= 1 and s % 2 == 0 and s // 2 + 1 < NBLK:
                        emit_gather(s // 2 + 1)
                    if s + 1 < NSEG:
                        nstate, pthunks = proj_thunks(s + 1)
                        seg_ps[s + 1] = nstate
                        pending_proj = pthunks
                    else:
                        pending_proj = []
                    if s >= 1:
                        pending_soft = softmax_thunks(
                            s - 1, seg_ps[s - 1]["ps"], st_state
                        )
                    else:
                        pending_soft = []

                    ps = seg_ps[s]["ps"]
                    for tau in range(SEG):
                        t = s * SEG + tau
                        csl = slice(tau * BC, (tau + 1) * BC)
                        if t > 0:
                            for kc in range(KC):
                                for ic in range(KC):
                                    nc.tensor.matmul(
                                        ps[:, ic, csl],
                                        whh_sb[:, kc, ic * P : (ic + 1) * P],
                                        hT_all[:, t - 1, kc, :],
                                        start=False,
                                        stop=(kc == KC - 1 and ic == KC - 1),
                                        skip_group_check=True,
                                    )
                        nc.scalar.activation(
                            hT_all[:, t, :, :], ps[:, :, csl], Act.Tanh
                        )
                        for _ in range(2):
                            if pending_proj:
                                pending_proj.pop(0)()
                        if pending_soft:
                            pending_soft.pop(0)()
                    for th in pending_proj:
                        th()
                    for th in pending_soft:
                        th()

            # ---- epilogue: softmax of the final segment -------------------
            with nc.named_scope("softmax_tail"):
                for th in softmax_thunks(NSEG - 1, seg_ps[NSEG - 1]["ps"], st_state):
                    th()

    nc.compile()
    return nc


def _prep_core_inputs(x_core, shared):
    flat = np.ascontiguousarray(x_core.T).reshape(-1).astype(np.int16)  # j = t*8+b
    idx = np.zeros((P, NIDX // 16), np.int16)
    for nb in range(NBLK):
        blk = flat[nb * BLK : (nb + 1) * BLK].reshape(BLK // 16, 16).T  # [16, 32]
        idx[:, nb * 32 : (nb + 1) * 32] = np.tile(blk, (P // 16, 1))
    m = dict(shared)
    m["idx"] = idx
    return m


def _hi_lo(a):
    hi = a.astype(ml_dtypes.bfloat16)
    lo = (a - hi.astype(np.float32)).astype(ml_dtypes.bfloat16)
    return hi, lo


def _shared_inputs(emb, W_ih, W_hh, b_ih, b_hh):
    emb_hi, emb_lo = _hi_lo(emb)
    embp = np.concatenate([emb_hi, emb_lo], axis=1)  # [VOCAB, 1024]
    wih_hi, wih_lo = _hi_lo(np.ascontiguousarray(W_ih.T))
    bias = (b_ih + b_hh).astype(np.float32)
    b_hi, b_lo = _hi_lo(bias)
    bh = b_hi.astype(np.float32).reshape(KC, P)
    bl = b_lo.astype(np.float32).reshape(KC, P)
    biasm = np.zeros((P, 2, P), np.float32)
    for bank in range(2):
        biasm[0, bank] = bh[2 * bank]
        biasm[1, bank] = bh[2 * bank + 1]
        biasm[2, bank] = bl[2 * bank]
        biasm[3, bank] = bl[2 * bank + 1]
    biasm = biasm.astype(ml_dtypes.bfloat16)
    e0 = np.zeros((P, 2 * SCOL), ml_dtypes.bfloat16)
    e0[0, :SCOL] = 1
    e0[1, SCOL:] = 1
    e0[2, :SCOL] = 1
    e0[3, SCOL:] = 1
    return {
        "embp": np.ascontiguousarray(embp),
        "wih_hi": np.ascontiguousarray(wih_hi),
        "wih_lo": np.ascontiguousarray(wih_lo),
        "whh": np.ascontiguousarray(W_hh.T).astype(ml_dtypes.bfloat16),
        "biasm": biasm,
        "e0": e0,
        "ones": np.ones((P, P), np.float32),
    }


def kernel(x, emb, W_ih, W_hh, b_ih, b_hh):
    global LAST_RESULT
    from concourse.bass_utils import run_bass_kernel_spmd

    x = np.asarray(x)
    emb = np.asarray(emb, dtype=np.float32)
    W_ih = np.asarray(W_ih, dtype=np.float32)
    W_hh = np.asarray(W_hh, dtype=np.float32)
    b_ih = np.asarray(b_ih, dtype=np.float32)
    b_hh = np.asarray(b_hh, dtype=np.float32)

    if "nc" not in _cache:
        _cache["nc"] = _build()
    nc = _cache["nc"]

    shared = _shared_inputs(emb, W_ih, W_hh, b_ih, b_hh)
    in_maps = [
        _prep_core_inputs(x[c * BC : (c + 1) * BC], shared) for c in range(NCORES)
    ]
    res = run_bass_kernel_spmd(
        nc, in_maps, core_ids=list(range(NCORES)), trace=TRACE,
        **({"stitch_traces": True} if TRACE else {}),
    )
    LAST_RESULT = res
    return np.concatenate([res.results[c]["out"] for c in range(NCORES)], axis=0)


# revision 13
# speedup vs baseline: 1.0805x; 1.0805x over previous
"""Trainium2 Bass kernel for nn_BaselineModel_47682726921062.

Model: token embedding lookup -> input projection -> 512-step tanh RNN
-> softmax over the hidden dim. Output [64, 512, 512] = softmax(h, axis=1)
with h[b, :, t] the hidden state after step t.

Strategy: data-parallel over batch across 8 NeuronCores (8 examples/core),
weights replicated, zero collectives.

v2 over the v1 baseline:
  - xp (input projection + bias) is accumulated DIRECTLY into PSUM via
    matmuls (bias by outer-product MM with a first-row-ones moving tile),
    and the recurrence matmuls accumulate onto it (start=False). This
    removes the per-step VectorE add from the serial chain entirely:
    per step only [16 MMs] -> Tanh -> [16 MMs] remains.
  - the sequence is processed in 16 segments of 32 steps; PSUM holds xp
    for 3 segments (2 banks each, 6 of 8 banks) so gather, projection
    (seg s+1) and softmax+output (seg s-1) all run in the engine idle
    slots under the recurrence of seg s.
  - numerics: embedding rows are gathered as host-precomputed (hi, lo)
    bf16 pairs (one 2KB-elem transposing SWDGE gather), W_ih applied as
    hi/lo bf16 pair (3 cross terms), bias as hi/lo pair, softmax wholly
    in fp32 from a second PSUM read (tanh_f32 -> exp -> ones-matmul sums
    -> DVE reciprocal -> fp32 normalize). Only W_hh and the recurrent h
    stay plain bf16.
"""

import sys

if "/opt/trn_rl_repo" not in sys.path:
    sys.path.insert(0, "/opt/trn_rl_repo")

import numpy as np
import ml_dtypes

BATCH, SEQ, VOCAB, DIM = 64, 512, 32000, 512
NCORES = 8
BC = BATCH // NCORES          # 8 examples per core
P = 128
KC = DIM // P                 # 4 chunks of 128
NIDX = SEQ * BC               # 4096 gathered rows per core
NBLK = 8                      # gather blocks of 512 (t,b) rows
BLK = NIDX // NBLK            # 512
SEG = 32                      # recurrence steps per segment
NSEG = SEQ // SEG             # 16
SCOL = SEG * BC               # 256 psum columns per segment

TRACE = False
LAST_RESULT = None

_cache = {}


def _build(repeat=1):
    import concourse.mybir as mybir
    import concourse.tile as tile
    from concourse import bacc

    f32 = mybir.dt.float32
    bf16 = mybir.dt.bfloat16
    Act = mybir.ActivationFunctionType

    nc = bacc.Bacc("TRN2")

    embp = nc.dram_tensor("embp", [VOCAB, 2 * DIM], bf16, kind="ExternalInput")
    idx = nc.dram_tensor("idx", [P, NIDX // 16], mybir.dt.int16, kind="ExternalInput")
    wih_hi = nc.dram_tensor("wih_hi", [DIM, DIM], bf16, kind="ExternalInput")  # W_ih.T hi
    wih_lo = nc.dram_tensor("wih_lo", [DIM, DIM], bf16, kind="ExternalInput")  # W_ih.T lo
    whh = nc.dram_tensor("whh", [DIM, DIM], bf16, kind="ExternalInput")        # W_hh.T
    # biasm[k, bank, m]: rows 0/1 = bias_hi for the bank's two mc halves,
    # rows 2/3 = bias_lo; one N=512 outer-product MM per PSUM bank writes
    # bias over the whole bank (the only start=True MM the bank ever sees).
    biasm = nc.dram_tensor("biasm", [P, 2, P], bf16, kind="ExternalInput")
    e0 = nc.dram_tensor("e0", [P, 2 * SCOL], bf16, kind="ExternalInput")
    ones = nc.dram_tensor("ones", [P, P], f32, kind="ExternalInput")
    out = nc.dram_tensor("out", [BC, DIM, SEQ], f32, kind="ExternalOutput")

    with tile.TileContext(nc) as tc:
        with (
            tc.tile_pool(name="consts", bufs=1) as consts,
            tc.tile_pool(name="xe", bufs=2) as xe_pool,
            tc.tile_pool(name="h", bufs=1) as h_pool,
            tc.tile_pool(name="tf", bufs=2) as tf_pool,
            tc.tile_pool(name="ex", bufs=2) as ex_pool,
            tc.tile_pool(name="rc", bufs=2) as rc_pool,
            tc.tile_pool(name="st", bufs=2) as st_pool,
            tc.tile_pool(name="xp", bufs=3, space="PSUM") as xp_pool,
            tc.tile_pool(name="sm", bufs=2, space="PSUM") as sm_pool,
        ):
            idx_sb = consts.tile([P, NIDX // 16], mybir.dt.int16)
            nc.sync.dma_start(idx_sb[:], idx[:])
            wih_hi_sb = consts.tile([P, KC, DIM], bf16)
            nc.sync.dma_start(wih_hi_sb[:], wih_hi.rearrange("(kc p) m -> p kc m", p=P))
            wih_lo_sb = consts.tile([P, KC, DIM], bf16)
            nc.sync.dma_start(wih_lo_sb[:], wih_lo.rearrange("(kc p) m -> p kc m", p=P))
            whh_sb = consts.tile([P, KC, DIM], bf16)
            nc.sync.dma_start(whh_sb[:], whh.rearrange("(kc p) m -> p kc m", p=P))
            biasm_sb = consts.tile([P, 2, P], bf16)
            nc.sync.dma_start(biasm_sb[:], biasm[:])
            e0_sb = consts.tile([P, 2 * SCOL], bf16)
            nc.sync.dma_start(e0_sb[:], e0[:])
            ones_sb = consts.tile([P, P], f32)
            nc.sync.dma_start(ones_sb[:], ones[:])

            hT_all = h_pool.tile([P, SEQ, KC, BC], bf16)   # 32 KB/partition

            xe_blocks = {}

            def emit_gather(nb):
                xe = xe_pool.tile([P, 2 * KC, BLK], bf16, tag="xe")
                nc.gpsimd.dma_gather(
                    xe[:], embp[:], idx_sb[:, nb * 32 : (nb + 1) * 32],
                    num_idxs=BLK, num_idxs_reg=BLK, elem_size=2 * DIM,
                    transpose=True,
                )
                xe_blocks[nb] = xe

            def block_of(gseg):
                return (gseg % NSEG) // 2

            def proj_thunks(s):
                """One thunk per matmul for segment s's xp preload."""
                nb, half = s // 2, s % 2
                col = slice(half * SCOL, (half + 1) * SCOL)
                state = {}

                def alloc():
                    state["ps"] = xp_pool.tile([P, KC, SCOL], f32, tag="xp", name="xp")

                def bias_mm(bank):
                    if bank == 0:
                        alloc()
                    # one start=True MM covering the entire PSUM bank
                    nc.tensor.matmul(
                        state["ps"][:, 2 * bank : 2 * bank + 2, :].rearrange(
                            "p c n -> p (c n)"
                        ),
                        biasm_sb[:, bank, :],
                        e0_sb[:],
                        start=True, stop=False, skip_group_check=True,
                    )

                thunks = [lambda: bias_mm(0), lambda: bias_mm(1)]
                for mc in range(KC):
                    msl = slice(mc * P, (mc + 1) * P)
                    for kc in range(KC):
                        for w_sb, xc in (
                            (wih_hi_sb, kc),          # Whi * xe_hi
                            (wih_hi_sb, KC + kc),     # Whi * xe_lo
                            (wih_lo_sb, kc),          # Wlo * xe_hi
                        ):
                            def pmm(mc=mc, msl=msl, kc=kc, w_sb=w_sb, xc=xc):
                                nc.tensor.matmul(
                                    state["ps"][:, mc, :],
                                    w_sb[:, kc, msl],
                                    xe_blocks[nb][:, xc, col],
                                    start=False, stop=False,
                                    skip_group_check=True,
                                )
                            thunks.append(pmm)
                return state, thunks

            def softmax_thunks(s, ps, st_state):
                """Softmax + output for segment s, reading its psum tile."""
                thunks = []
                state = {}

                def mk_tf(mc):
                    if mc == 0:
                        state["tf"] = tf_pool.tile([P, KC, SCOL], f32, tag="tf", name="tf")
                    nc.scalar.activation(state["tf"][:, mc, :], ps[:, mc, :], Act.Tanh)

                def mk_ex(mc):
                    if mc == 0:
                        state["ex"] = ex_pool.tile([P, KC, SCOL], f32, tag="ex", name="ex")
                    nc.scalar.activation(state["ex"][:, mc, :], state["tf"][:, mc, :], Act.Exp)

                def mk_sum(mc):
                    if mc == 0:
                        state["sp"] = sm_pool.tile([P, SCOL], f32, tag="sum", name="sp")
                    nc.tensor.matmul(
                        state["sp"][:], ones_sb[:], state["ex"][:, mc, :],
                        start=(mc == 0), stop=(mc == KC - 1),
                        skip_group_check=True,
                    )

                def mk_recip():
                    state["rc"] = rc_pool.tile([P, SEG, BC], f32, tag="rc", name="rc")
                    nc.vector.reciprocal(
                        state["rc"][:], state["sp"][:].rearrange("p (t b) -> p t b", b=BC)
                    )

                def mk_norm(mc):
                    if mc == 0 and s % 2 == 0:
                        st_state["st"] = st_pool.tile([P, KC, BC, 2 * SEG], f32, tag="st", name="st")
                    toff = (s % 2) * SEG
                    nc.vector.tensor_tensor(
                        st_state["st"][:, mc, :, toff : toff + SEG].rearrange(
                            "p b t -> p t b"
                        ),
                        state["ex"][:, mc, :].rearrange("p (t b) -> p t b", b=BC),
                        state["rc"][:],
                        mybir.AluOpType.mult,
                    )

                def mk_dma(mc):
                    ts0 = (s // 2) * 2 * SEG
                    nc.sync.dma_start(
                        out[:, mc * P : (mc + 1) * P, ts0 : ts0 + 2 * SEG].rearrange(
                            "b p t -> p b t"
                        ),
                        st_state["st"][:, mc],
                    )

                for mc in range(KC):
                    thunks.append(lambda mc=mc: mk_tf(mc))
                for mc in range(KC):
                    thunks.append(lambda mc=mc: mk_ex(mc))
                for mc in range(KC):
                    thunks.append(lambda mc=mc: mk_sum(mc))
                thunks.append(mk_recip)
                for mc in range(KC):
                    thunks.append(lambda mc=mc: mk_norm(mc))
                if s % 2 == 1:
                    for mc in range(KC):
                        thunks.append(lambda mc=mc: mk_dma(mc))
                return thunks

            # ---- prologue -------------------------------------------------
            emit_gather(0)
            st_state = {}
            proj_state, thunks0 = proj_thunks(0)
            for th in thunks0:
                th()
            seg_ps = {0: proj_state}

            pending_proj = []
            pending_soft = []

            with nc.named_scope("recurrence"):
                for gs in range(repeat * NSEG):
                    s = gs % NSEG
                    # gather 2 segments ahead of the proj that consumes it
                    if gs % 2 == 0 and gs + 2 <= repeat * NSEG - 1:
                        emit_gather(block_of(gs + 2))
                    if gs + 1 < repeat * NSEG:
                        nstate, pthunks = proj_thunks((gs + 1) % NSEG)
                        seg_ps[gs + 1] = nstate
                        pending_proj = pthunks
                    else:
                        pending_proj = []
                    if gs >= 1:
                        pending_soft = softmax_thunks(
                            (gs - 1) % NSEG, seg_ps[gs - 1]["ps"], st_state
                        )
                    else:
                        pending_soft = []

                    ps = seg_ps[gs]["ps"]
                    for tau in range(SEG):
                        t = s * SEG + tau
                        csl = slice(tau * BC, (tau + 1) * BC)
                        if t > 0:
                            for kc in range(KC):
                                for ic in range(KC):
                                    nc.tensor.matmul(
                                        ps[:, ic, csl],
                                        whh_sb[:, kc, ic * P : (ic + 1) * P],
                                        hT_all[:, t - 1, kc, :],
                                        start=False,
                                        stop=(kc == KC - 1 and ic == KC - 1),
                                        skip_group_check=True,
                                    )
                        nc.scalar.activation(
                            hT_all[:, t, :, :], ps[:, :, csl], Act.Tanh
                        )
                        for _ in range(2):
                            if pending_proj:
                                pending_proj.pop(0)()
                        if pending_soft:
                            pending_soft.pop(0)()
                    for th in pending_proj:
                        th()
                    for th in pending_soft:
                        th()

            # ---- epilogue: softmax of the final segment -------------------
            with nc.named_scope("softmax_tail"):
                last = repeat * NSEG - 1
                for th in softmax_thunks(NSEG - 1, seg_ps[last]["ps"], st_state):
                    th()

    nc.compile()
    return nc


def _prep_core_inputs(x_core, shared):
    flat = np.ascontiguousarray(x_core.T).reshape(-1).astype(np.int16)  # j = t*8+b
    idx = np.zeros((P, NIDX // 16), np.int16)
    for nb in range(NBLK):
        blk = flat[nb * BLK : (nb + 1) * BLK].reshape(BLK // 16, 16).T  # [16, 32]
        idx[:, nb * 32 : (nb + 1) * 32] = np.tile(blk, (P // 16, 1))
    m = dict(shared)
    m["idx"] = idx
    return m


def _hi_lo(a):
    hi = a.astype(ml_dtypes.bfloat16)
    lo = (a - hi.astype(np.float32)).astype(ml_dtypes.bfloat16)
    return hi, lo


def _shared_inputs(emb, W_ih, W_hh, b_ih, b_hh):
    emb_hi, emb_lo = _hi_lo(emb)
    embp = np.concatenate([emb_hi, emb_lo], axis=1)  # [VOCAB, 1024]
    wih_hi, wih_lo = _hi_lo(np.ascontiguousarray(W_ih.T))
    bias = (b_ih + b_hh).astype(np.float32)
    b_hi, b_lo = _hi_lo(bias)
    bh = b_hi.astype(np.float32).reshape(KC, P)
    bl = b_lo.astype(np.float32).reshape(KC, P)
    biasm = np.zeros((P, 2, P), np.float32)
    for bank in range(2):
        biasm[0, bank] = bh[2 * bank]
        biasm[1, bank] = bh[2 * bank + 1]
        biasm[2, bank] = bl[2 * bank]
        biasm[3, bank] = bl[2 * bank + 1]
    biasm = biasm.astype(ml_dtypes.bfloat16)
    e0 = np.zeros((P, 2 * SCOL), ml_dtypes.bfloat16)
    e0[0, :SCOL] = 1
    e0[1, SCOL:] = 1
    e0[2, :SCOL] = 1
    e0[3, SCOL:] = 1
    return {
        "embp": np.ascontiguousarray(embp),
        "wih_hi": np.ascontiguousarray(wih_hi),
        "wih_lo": np.ascontiguousarray(wih_lo),
        "whh": np.ascontiguousarray(W_hh.T).astype(ml_dtypes.bfloat16),
        "biasm": biasm,
        "e0": e0,
        "ones": np.ones((P, P), np.float32),
    }


def kernel(x, emb, W_ih, W_hh, b_ih, b_hh):
    global LAST_RESULT
    from concourse.bass_utils import run_bass_kernel_spmd

    x = np.asarray(x)
    emb = np.asarray(emb, dtype=np.float32)
    W_ih = np.asarray(W_ih, dtype=np.float32)
    W_hh = np.asarray(W_hh, dtype=np.float32)
    b_ih = np.asarray(b_ih, dtype=np.float32)
    b_hh = np.asarray(b_hh, dtype=np.float32)

    if "nc" not in _cache:
        _cache["nc"] = _build()
    nc = _cache["nc"]

    shared = _shared_inputs(emb, W_ih, W_hh, b_ih, b_hh)
    in_maps = [
        _prep_core_inputs(x[c * BC : (c + 1) * BC], shared) for c in range(NCORES)
    ]
    res = run_bass_kernel_spmd(
        nc, in_maps, core_ids=list(range(NCORES)), trace=TRACE,
        **({"stitch_traces": True} if TRACE else {}),
    )
    LAST_RESULT = res
    return np.concatenate([res.results[c]["out"] for c in range(NCORES)], axis=0)


# revision 28
# speedup vs baseline: 1.3673x; 1.2654x over previous
"""Trainium2 Bass kernel for nn_BaselineModel_47682726921062.

Model: token embedding lookup -> input projection -> 512-step tanh RNN
-> softmax over the hidden dim. Output [64, 512, 512] = softmax(h, axis=1)
with h[b, :, t] the hidden state after step t.

Strategy: data-parallel over batch across 8 NeuronCores (8 examples/core),
weights replicated, zero collectives.

v2 over the v1 baseline:
  - xp (input projection + bias) is accumulated DIRECTLY into PSUM via
    matmuls (bias by outer-product MM with a first-row-ones moving tile),
    and the recurrence matmuls accumulate onto it (start=False). This
    removes the per-step VectorE add from the serial chain entirely:
    per step only [16 MMs] -> Tanh -> [16 MMs] remains.
  - the sequence is processed in 16 segments of 32 steps; PSUM holds xp
    for 3 segments (2 banks each, 6 of 8 banks) so gather, projection
    (seg s+1) and softmax+output (seg s-1) all run in the engine idle
    slots under the recurrence of seg s.
  - numerics: embedding rows are gathered as host-precomputed (hi, lo)
    bf16 pairs (one 2KB-elem transposing SWDGE gather), W_ih applied as
    hi/lo bf16 pair (3 cross terms), bias as hi/lo pair, softmax wholly
    in fp32 from a second PSUM read (tanh_f32 -> exp -> ones-matmul sums
    -> DVE reciprocal -> fp32 normalize). Only W_hh and the recurrent h
    stay plain bf16.
"""

import sys

if "/opt/trn_rl_repo" not in sys.path:
    sys.path.insert(0, "/opt/trn_rl_repo")

import numpy as np
import ml_dtypes

BATCH, SEQ, VOCAB, DIM = 64, 512, 32000, 512
NCORES = 8
BC = BATCH // NCORES          # 8 examples per core
P = 128
KC = DIM // P                 # 4 chunks of 128
NIDX = SEQ * BC               # 4096 gathered rows per core
NBLK = 8                      # gather blocks of 512 (t,b) rows
BLK = NIDX // NBLK            # 512
SEG = 32                      # recurrence steps per segment
NSEG = SEQ // SEG             # 16
SCOL = SEG * BC               # 256 psum columns per segment

TRACE = False
LAST_RESULT = None

_cache = {}


def _build(repeat=1):
    import concourse.mybir as mybir
    import concourse.tile as tile
    from concourse import bacc

    f32 = mybir.dt.float32
    bf16 = mybir.dt.bfloat16
    Act = mybir.ActivationFunctionType

    nc = bacc.Bacc("TRN2")

    embp = nc.dram_tensor("embp", [VOCAB, 2 * DIM], bf16, kind="ExternalInput")
    idx = nc.dram_tensor("idx", [P, NIDX // 16], mybir.dt.int16, kind="ExternalInput")
    wih_hi = nc.dram_tensor("wih_hi", [DIM, DIM], bf16, kind="ExternalInput")  # W_ih.T hi
    wih_lo = nc.dram_tensor("wih_lo", [DIM, DIM], bf16, kind="ExternalInput")  # W_ih.T lo
    whh = nc.dram_tensor("whh", [DIM, DIM], bf16, kind="ExternalInput")        # W_hh.T
    # biasm[k, bank, m]: rows 0/1 = bias_hi for the bank's two mc halves,
    # rows 2/3 = bias_lo; one N=512 outer-product MM per PSUM bank writes
    # bias over the whole bank (the only start=True MM the bank ever sees).
    biasm = nc.dram_tensor("biasm", [P, 2, P], bf16, kind="ExternalInput")
    e0 = nc.dram_tensor("e0", [P, 2 * SCOL], bf16, kind="ExternalInput")
    ones = nc.dram_tensor("ones", [P, P], f32, kind="ExternalInput")
    out = nc.dram_tensor("out", [BC, DIM, SEQ], f32, kind="ExternalOutput")

    with tile.TileContext(nc) as tc:
        with (
            tc.tile_pool(name="consts", bufs=1) as consts,
            tc.tile_pool(name="xe", bufs=2) as xe_pool,
            tc.tile_pool(name="h", bufs=1) as h_pool,
            tc.tile_pool(name="tf", bufs=2) as tf_pool,
            tc.tile_pool(name="ex", bufs=2) as ex_pool,
            tc.tile_pool(name="rc", bufs=2) as rc_pool,
            tc.tile_pool(name="st", bufs=2) as st_pool,
            tc.tile_pool(name="xp", bufs=2, space="PSUM") as xp_pool,
            tc.tile_pool(name="sm", bufs=2, space="PSUM") as sm_pool,
        ):
            idx_sb = consts.tile([P, NIDX // 16], mybir.dt.int16)
            nc.sync.dma_start(idx_sb[:], idx[:])

            hT_all = h_pool.tile([P, SEQ, KC, BC], bf16)   # 32 KB/partition

            xe_blocks = {}

            def emit_gather(nb):
                xe = xe_pool.tile([P, 2 * KC, BLK], bf16, tag="xe")
                nc.gpsimd.dma_gather(
                    xe[:], embp[:], idx_sb[:, nb * 32 : (nb + 1) * 32],
                    num_idxs=BLK, num_idxs_reg=BLK, elem_size=2 * DIM,
                    transpose=True,
                )
                xe_blocks[nb] = xe

            def block_of(gseg):
                return (gseg % NSEG) // 2

            # gather for segment 0 first: it only needs idx_sb, and jumping
            # the const-DMA queue takes it off the prologue critical path
            emit_gather(0)

            wih_hi_sb = consts.tile([P, KC, DIM], bf16)
            nc.sync.dma_start(wih_hi_sb[:], wih_hi.rearrange("(kc p) m -> p kc m", p=P))
            wih_lo_sb = consts.tile([P, KC, DIM], bf16)
            nc.sync.dma_start(wih_lo_sb[:], wih_lo.rearrange("(kc p) m -> p kc m", p=P))
            biasm_sb = consts.tile([P, 2, P], bf16)
            nc.sync.dma_start(biasm_sb[:], biasm[:])
            e0_sb = consts.tile([P, 2 * SCOL], bf16)
            nc.sync.dma_start(e0_sb[:], e0[:])
            whh_sb = consts.tile([P, KC, DIM], bf16)
            nc.sync.dma_start(whh_sb[:], whh.rearrange("(kc p) m -> p kc m", p=P))
            ones_sb = consts.tile([P, P], f32)
            nc.sync.dma_start(ones_sb[:], ones[:])

            def proj_thunks(s):
                """One thunk per matmul for segment s's xp preload."""
                nb, half = s // 2, s % 2
                col = slice(half * SCOL, (half + 1) * SCOL)
                state = {}

                def alloc():
                    state["ps"] = xp_pool.tile([P, KC, SCOL], f32, tag="xp", name="xp")

                def bias_mm(bank):
                    if bank == 0:
                        alloc()
                    # one start=True MM covering the entire PSUM bank
                    nc.tensor.matmul(
                        state["ps"][:, 2 * bank : 2 * bank + 2, :].rearrange(
                            "p c n -> p (c n)"
                        ),
                        biasm_sb[:, bank, :],
                        e0_sb[:],
                        start=True, stop=False, skip_group_check=True,
                    )

                thunks = [lambda: bias_mm(0), lambda: bias_mm(1)]
                for mc in range(KC):
                    msl = slice(mc * P, (mc + 1) * P)
                    for kc in range(KC):
                        for w_sb, xc in (
                            (wih_hi_sb, kc),          # Whi * xe_hi
                            (wih_hi_sb, KC + kc),     # Whi * xe_lo
                            (wih_lo_sb, kc),          # Wlo * xe_hi
                        ):
                            def pmm(mc=mc, msl=msl, kc=kc, w_sb=w_sb, xc=xc):
                                nc.tensor.matmul(
                                    state["ps"][:, mc, :],
                                    w_sb[:, kc, msl],
                                    xe_blocks[nb][:, xc, col],
                                    start=False, stop=False,
                                    skip_group_check=True,
                                )
                            thunks.append(pmm)
                return state, thunks

            def softmax_piece(t0, nsteps, toff, alloc_st, dma_ts0, dma_nt):
                """Softmax + output for hT steps [t0, t0+nsteps); writes st
                cols [toff, toff+nsteps); emits the out DMA for t-range
                [dma_ts0, dma_ts0+dma_nt) if dma_ts0 is not None."""
                thunks = []
                state = {}
                ncol = nsteps * BC

                def mk_ex(mc):
                    if mc == 0:
                        state["ex"] = ex_pool.tile([P, KC, SCOL], f32, tag="ex", name="ex", bufs=2)
                    nc.scalar.activation(
                        state["ex"][:, mc, :ncol].rearrange("p (t b) -> p t b", b=BC),
                        hT_all[:, t0 : t0 + nsteps, mc, :],
                        Act.Exp,
                    )

                def mk_sum(mc):
                    if mc == 0:
                        state["sp"] = sm_pool.tile([P, SCOL], f32, tag="sum", name="sp", bufs=2)
                    nc.tensor.matmul(
                        state["sp"][:, :ncol], ones_sb[:], state["ex"][:, mc, :ncol],
                        start=(mc == 0), stop=(mc == KC - 1),
                        skip_group_check=True,
                    )

                def mk_recip():
                    state["rc"] = rc_pool.tile([P, SEG, BC], f32, tag="rc", name="rc", bufs=2)
                    nc.vector.reciprocal(
                        state["rc"][:, :nsteps, :],
                        state["sp"][:, :ncol].rearrange("p (t b) -> p t b", b=BC),
                    )

                def mk_norm(mc):
                    if mc == 0 and alloc_st:
                        st_state["st"] = st_pool.tile([P, KC, BC, 2 * SEG], f32, tag="st", name="st")
                    nc.vector.tensor_tensor(
                        st_state["st"][:, mc, :, toff : toff + nsteps].rearrange(
                            "p b t -> p t b"
                        ),
                        state["ex"][:, mc, :ncol].rearrange("p (t b) -> p t b", b=BC),
                        state["rc"][:, :nsteps, :],
                        mybir.AluOpType.mult,
                    )

                def mk_dma(mc):
                    st_off = dma_ts0 % (2 * SEG)
                    nc.sync.dma_start(
                        out[:, mc * P : (mc + 1) * P, dma_ts0 : dma_ts0 + dma_nt].rearrange(
                            "b p t -> p b t"
                        ),
                        st_state["st"][:, mc, :, st_off : st_off + dma_nt],
                    )

                for mc in range(KC):
                    thunks.append(lambda mc=mc: mk_ex(mc))
                for mc in range(KC):
                    thunks.append(lambda mc=mc: mk_sum(mc))
                thunks.append(mk_recip)
                for mc in range(KC):
                    thunks.append(lambda mc=mc: mk_norm(mc))
                if dma_ts0 is not None:
                    for mc in range(KC):
                        thunks.append(lambda mc=mc: mk_dma(mc))
                return thunks

            def softmax_thunks(s, t0, st_state):
                """Softmax + output for segment s (steady-state path)."""
                return softmax_piece(
                    t0, SEG, (s % 2) * SEG, alloc_st=(s % 2 == 0),
                    dma_ts0=t0, dma_nt=SEG,
                )

            # ---- prologue -------------------------------------------------
            st_state = {}
            proj_state, thunks0 = proj_thunks(0)
            for th in thunks0:
                th()
            seg_ps = {0: proj_state}

            pending_proj = []
            pending_soft = []

            with nc.named_scope("recurrence"):
                for gs in range(repeat * NSEG):
                    s = gs % NSEG
                    # gather 2 segments ahead of the proj that consumes it
                    if gs % 2 == 0 and gs + 2 <= repeat * NSEG - 1:
                        emit_gather(block_of(gs + 2))
                    if gs + 1 < repeat * NSEG:
                        nstate, pthunks = proj_thunks((gs + 1) % NSEG)
                        seg_ps[gs + 1] = nstate
                        pending_proj = pthunks
                    else:
                        pending_proj = []
                    if gs >= 1:
                        sprev = (gs - 1) % NSEG
                        pending_soft = softmax_thunks(sprev, sprev * SEG, st_state)
                    else:
                        pending_soft = []

                    ps = seg_ps[gs]["ps"]
                    for tau in range(SEG):
                        t = s * SEG + tau
                        csl = slice(tau * BC, (tau + 1) * BC)
                        if t > 0:
                            for kc in range(KC):
                                for ic in range(KC):
                                    nc.tensor.matmul(
                                        ps[:, ic, csl],
                                        whh_sb[:, kc, ic * P : (ic + 1) * P],
                                        hT_all[:, t - 1, kc, :],
                                        start=False,
                                        stop=(kc == KC - 1 and ic == KC - 1),
                                        skip_group_check=True,
                                    )
                        nc.scalar.activation(
                            hT_all[:, t, :, :], ps[:, :, csl], Act.Tanh
                        )
                        for _ in range(2):
                            if pending_proj:
                                pending_proj.pop(0)()
                        if pending_soft:
                            pending_soft.pop(0)()
                    for th in pending_proj:
                        th()
                    for th in pending_soft:
                        th()

            # ---- epilogue: softmax of the final segment, in two 16-step
            # pieces so the scheduler can hoist the first under the last
            # recurrence steps --------------------------------------------
            with nc.named_scope("softmax_tail"):
                t0 = (NSEG - 1) * SEG
                half = SEG // 2
                for th in softmax_piece(t0, half, SEG, False, t0, half):
                    th()
                for th in softmax_piece(
                    t0 + half, half, SEG + half, False, t0 + half, half
                ):
                    th()

    nc.compile()
    return nc


def _prep_core_inputs(x_core, shared):
    flat = np.ascontiguousarray(x_core.T).reshape(-1).astype(np.int16)  # j = t*8+b
    idx = np.zeros((P, NIDX // 16), np.int16)
    for nb in range(NBLK):
        blk = flat[nb * BLK : (nb + 1) * BLK].reshape(BLK // 16, 16).T  # [16, 32]
        idx[:, nb * 32 : (nb + 1) * 32] = np.tile(blk, (P // 16, 1))
    m = dict(shared)
    m["idx"] = idx
    return m


def _hi_lo(a):
    hi = a.astype(ml_dtypes.bfloat16)
    lo = (a - hi.astype(np.float32)).astype(ml_dtypes.bfloat16)
    return hi, lo


def _shared_inputs(emb, W_ih, W_hh, b_ih, b_hh):
    emb_hi, emb_lo = _hi_lo(emb)
    embp = np.concatenate([emb_hi, emb_lo], axis=1)  # [VOCAB, 1024]
    wih_hi, wih_lo = _hi_lo(np.ascontiguousarray(W_ih.T))
    bias = (b_ih + b_hh).astype(np.float32)
    b_hi, b_lo = _hi_lo(bias)
    bh = b_hi.astype(np.float32).reshape(KC, P)
    bl = b_lo.astype(np.float32).reshape(KC, P)
    biasm = np.zeros((P, 2, P), np.float32)
    for bank in range(2):
        biasm[0, bank] = bh[2 * bank]
        biasm[1, bank] = bh[2 * bank + 1]
        biasm[2, bank] = bl[2 * bank]
        biasm[3, bank] = bl[2 * bank + 1]
    biasm = biasm.astype(ml_dtypes.bfloat16)
    e0 = np.zeros((P, 2 * SCOL), ml_dtypes.bfloat16)
    e0[0, :SCOL] = 1
    e0[1, SCOL:] = 1
    e0[2, :SCOL] = 1
    e0[3, SCOL:] = 1
    return {
        "embp": np.ascontiguousarray(embp),
        "wih_hi": np.ascontiguousarray(wih_hi),
        "wih_lo": np.ascontiguousarray(wih_lo),
        "whh": np.ascontiguousarray(W_hh.T).astype(ml_dtypes.bfloat16),
        "biasm": biasm,
        "e0": e0,
        "ones": np.ones((P, P), np.float32),
    }


def kernel(x, emb, W_ih, W_hh, b_ih, b_hh):
    global LAST_RESULT
    from concourse.bass_utils import run_bass_kernel_spmd

    x = np.asarray(x)
    emb = np.asarray(emb, dtype=np.float32)
    W_ih = np.asarray(W_ih, dtype=np.float32)
    W_hh = np.asarray(W_hh, dtype=np.float32)
    b_ih = np.asarray(b_ih, dtype=np.float32)
    b_hh = np.asarray(b_hh, dtype=np.float32)

    if "nc" not in _cache:
        _cache["nc"] = _build()
    nc = _cache["nc"]

    shared = _shared_inputs(emb, W_ih, W_hh, b_ih, b_hh)
    in_maps = [
        _prep_core_inputs(x[c * BC : (c + 1) * BC], shared) for c in range(NCORES)
    ]
    res = run_bass_kernel_spmd(
        nc, in_maps, core_ids=list(range(NCORES)), trace=TRACE,
        **({"stitch_traces": True} if TRACE else {}),
    )
    LAST_RESULT = res
    return np.concatenate([res.results[c]["out"] for c in range(NCORES)], axis=0)
